# revision 24
# baseline (speedup 1.0000x reference)
"""BasisNetwork (continuous-conv GNN) on 8 Trainium2 NeuronCores.

Per layer (nodes dest-sharded across cores, all compute in bf16/psum-f32):
  out[i] = sum_{e->i} (phi[e] (x) x[j_e]) @ Wflat  +  x[i] @ fc_w + fc_b
Edges are host-packed into groups of <=8 dest nodes / <=128 edge slots; a
static block-diagonal "Sphi" matrix (Sphi[k, b*8+slot] = phi[e_k, b] *
[dest(e_k)==slot], b-major columns) lives in SBUF for all 4 layers.  One
matmul per group with the gathered x_j tile as the stationary operand:
  M[c, b*8+slot] = sum_k xj[k, c] * Sphi[k, b*8+slot]
PSUM M-tiles are evicted into a b-major SBUF buffer (m_b[c, b*256 + g*8+s])
so the node-side GEMM per 32-group super-batch (256 slots) streams a fully
CONTIGUOUS moving operand (a strided rhs streams ~2x slower on the PE):
  out^T[f, slot] = sum_b conv_w[l][b].T @ m_b[:, b*256:(b+1)*256] (+fc+bias)
The 16 conv matmuls round-robin over 4 PSUM accumulation chains in four
SEPARATE PSUM banks: consecutive matmuls never target the same bank region
(same-region accumulation serializes at ~460ns/matmul, and interleaved
start/stop groups within one bank zero-region are illegal); chains are
summed on ACT/DVE with bias/residual fused.  Activations are all-gathered
(bf16) between layers and expanded into a 256B-strided table for GPSIMD
dma_gather.  Gathers use 512-index chunks round-robined over all 4 SWDGE
queues (each queue drains on ~one SDMA engine at ~27GiB/s — small chunks
keep 2-3 chunks resident per ring so drains run back-to-back) with a
24-deep xq tile pool so the gather stream never waits on compute.
"""

import numpy as np

# ---------------- problem constants (hardcoded per contract) ----------------
N_NODES = 20000
F = 32               # feature width, all layers
NB = 4
B = NB * NB          # 16 basis functions
N_LAYERS = 4
OUT_SCALE = 1.0 / 128.0
NCORE = 8
NSH = N_NODES // NCORE       # 2500 dest nodes per core
GS = 8                       # node slots per group
GE = 128                     # edge slots per group
GCOL = GS * B                # 128 columns per group (slot*16+b)
EP = 128                     # padded row length (256B in bf16) for dma_gather
GCHUNK = 4                   # groups per gather chunk (512 idxs)
SUP = 8                      # chunks per node-GEMM super-batch (256 slots)
PROFILE = False              # time warm executes
LAST_EXEC_NS = None
LAST_TRACE = None


def _hat(x, n):
    c = np.linspace(-1.0, 1.0, n, dtype=np.float32)
    r = np.abs(x[:, None] - c[None, :]) * ((n - 1) * 0.5)
    return np.maximum(1.0 - r, 0.0).astype(np.float32)


def _edge_basis(edge_attr, edge_i, edge_j):
    d = np.clip(edge_attr.astype(np.float32), -1.0, 1.0)
    phi = (_hat(d[:, 0], NB)[:, :, None] * _hat(d[:, 1], NB)[:, None, :]).reshape(-1, B)
    phi *= (edge_i != edge_j).astype(np.float32)[:, None]
    return phi  # [E, B]


def _pack_core(degrees, node_ids):
    """Best-fit-decreasing bin packing: bins of <=GS nodes / <=GE edges."""
    order = np.argsort(-degrees, kind="stable")
    bins = []            # list of [edges, [node,...]]
    buckets = [[] for _ in range(GE + 1)]
    for oi in order:
        d = int(degrees[oi])
        n = int(node_ids[oi])
        placed = False
        for r in range(d, GE + 1):
            while buckets[r]:
                bi = buckets[r][-1]
                e_cnt, nodes = bins[bi]
                if GE - e_cnt != r or len(nodes) >= GS:
                    buckets[r].pop()
                    continue
                nodes.append(n)
                bins[bi][0] = e_cnt + d
                buckets[r].pop()
                nr = GE - bins[bi][0]
                if len(nodes) < GS and nr > 0:
                    buckets[nr].append(bi)
                placed = True
                break
            if placed:
                break
        if not placed:
            bins.append([d, [n]])
            bi = len(bins) - 1
            nr = GE - d
            if nr > 0:
                buckets[nr].append(bi)
    return [nodes for _, nodes in bins]


def _preprocess(edge_i, edge_j, edge_attr):
    ei = np.asarray(edge_i).astype(np.int64)
    ej = np.asarray(edge_j).astype(np.int64)
    phi = _edge_basis(np.asarray(edge_attr), ei, ej)

    deg = np.bincount(ei, minlength=N_NODES)
    packs = []
    ng_max = 0
    for c in range(NCORE):
        nid = np.arange(c * NSH, (c + 1) * NSH)
        groups = _pack_core(deg[nid], nid)
        packs.append(groups)
        ng_max = max(ng_max, len(groups))

    NG = -(-ng_max // (GCHUNK * SUP)) * (GCHUNK * SUP)   # multiple of 64
    NSLOT = NG * GS
    assert NCORE * NSLOT < 32768    # int16 gather indices

    slot_of_node = np.zeros(N_NODES, dtype=np.int64)
    for c in range(NCORE):
        for gi, nodes in enumerate(packs[c]):
            for s, n in enumerate(nodes):
                slot_of_node[n] = c * NSLOT + gi * GS + s

    # per-core tables
    sphi = np.zeros((NCORE, NG, GE, GCOL), dtype=np.float32)
    gsrc = np.zeros((NCORE, GE, NG), dtype=np.int16)   # [edge slot p, group] -> src row
    order_all = np.argsort(ei, kind="stable")
    starts = np.zeros(N_NODES + 1, dtype=np.int64)
    np.cumsum(deg, out=starts[1:])
    for c in range(NCORE):
        for gi, nodes in enumerate(packs[c]):
            k = 0
            for s, n in enumerate(nodes):
                for e in order_all[starts[n]:starts[n + 1]]:
                    sphi[c, gi, k, s::GS] = phi[e]
                    gsrc[c, k, gi] = slot_of_node[ej[e]]
                    k += 1
            assert k <= GE
    # wrapped gather-index layout: stream k = gi*128 + p -> [k%16, k//16], x8 cores
    idx_w = np.zeros((NCORE, 16, NG * GE // 16), dtype=np.int16)
    for c in range(NCORE):
        stream = gsrc[c].T.reshape(-1)          # k = gi*128 + p
        idx_w[c, :, :] = stream.reshape(-1, 16).T
    idx_w = np.tile(idx_w, (1, 8, 1))           # [NCORE, 128, NG*8]

    return NG, NSLOT, slot_of_node, sphi, idx_w


def _build_and_run(pre, fluidFeatures, conv_ws, fc_ws, fc_bs):
    import ml_dtypes
    import concourse.bass as bass
    import concourse.bacc as bacc
    import concourse.mybir as mybir
    import concourse.tile as tile
    from concourse.bass_utils import run_bass_kernel_spmd
    from concourse.masks import make_identity
    from concourse import library_config

    NG, NSLOT, slot_of_node, sphi, idx_w = pre
    NCHUNK = NG // GCHUNK
    NSUPER = NCHUNK // SUP
    NROWS = NCORE * NSLOT
    NSPHI_PIECES = 8

    bf16 = mybir.dt.bfloat16
    f32 = mybir.dt.float32
    i16 = mybir.dt.int16

    x0 = np.asarray(fluidFeatures, dtype=np.float32)
    x0_slots = np.zeros((NROWS, F), dtype=np.float32)
    x0_slots[slot_of_node] = x0
    x0_pad = np.zeros((NROWS, EP), dtype=np.float32)
    x0_pad[:, :F] = x0_slots

    conv_w = np.stack([np.asarray(w, dtype=np.float32) for w in conv_ws])  # [L,B,F,F]
    fc_w = np.stack([np.asarray(w, dtype=np.float32) for w in fc_ws])      # [L,F,F]
    fc_b = np.stack([np.asarray(b, dtype=np.float32) for b in fc_bs])      # [L,F]

    NS = SUP * GCHUNK * GS          # slots per super (128)

    # ------------------------------ build graph ------------------------------
    nc = bacc.Bacc("TRN2", target_bir_lowering=False, debug=False, num_devices=NCORE,
                   num_swdge_queues=4)

    d_x0pad = nc.dram_tensor("x0pad", [NROWS, EP], bf16, kind="ExternalInput")
    d_x0T = nc.dram_tensor("x0T", [F, NSLOT], bf16, kind="ExternalInput")
    d_sphi = nc.dram_tensor("sphi", [GE, NG * GCOL], bf16, kind="ExternalInput")
    d_gidx = nc.dram_tensor("gidx", [128, NG * GE // 16], i16, kind="ExternalInput")
    d_convw = nc.dram_tensor("convw", [F, N_LAYERS * B * F], bf16, kind="ExternalInput")
    d_fcw = nc.dram_tensor("fcw", [F, N_LAYERS * F], bf16, kind="ExternalInput")
    d_fcb = nc.dram_tensor("fcb", [F, N_LAYERS], f32, kind="ExternalInput")
    d_out = nc.dram_tensor("out", [NSLOT, F], f32, kind="ExternalOutput")
    d_ash = nc.dram_tensor("a_shard", [NSLOT, F], bf16, kind="Internal")
    d_afull0 = nc.dram_tensor("a_full0", [NROWS, F], bf16,
                              kind="Internal", addr_space="Shared")
    d_apad = nc.dram_tensor("a_pad", [NROWS, EP], bf16, kind="Internal")

    with tile.TileContext(nc) as tc:
        with (
            tc.tile_pool(name="persist", bufs=1) as pp,
            tc.tile_pool(name="xj", bufs=24) as xp,
            tc.tile_pool(name="mb", bufs=2) as mp,
            tc.tile_pool(name="work", bufs=2) as wp,
            tc.tile_pool(name="psc", bufs=2, space="PSUM") as psc,
            tc.tile_pool(name="pout", bufs=1, space="PSUM") as pout,
            tc.tile_pool(name="ptr", bufs=1, space="PSUM") as ptr,
        ):
            nc.gpsimd.load_library(library_config.mlp)

            gidx_sb = pp.tile([128, NG * GE // 16], i16)
            nc.sync.dma_start(out=gidx_sb[:], in_=d_gidx[:])
            convw_sb = pp.tile([F, N_LAYERS * B * F], bf16)
            nc.sync.dma_start(out=convw_sb[:], in_=d_convw[:])
            fcw_sb = pp.tile([F, N_LAYERS * F], bf16)
            nc.sync.dma_start(out=fcw_sb[:], in_=d_fcw[:])
            fcb_sb = pp.tile([F, N_LAYERS], f32)
            nc.sync.dma_start(out=fcb_sb[:], in_=d_fcb[:])
            id_bf = pp.tile([F, F], bf16)
            make_identity(nc, id_bf[:])
            id_f32 = pp.tile([F, F], f32)
            make_identity(nc, id_f32[:])

            # resident Sphi: loaded once in pieces, reused by all layers
            sphi_sb = pp.tile([GE, NG * GCOL], bf16)
            PIECE = NG * GCOL // NSPHI_PIECES
            for pi in range(NSPHI_PIECES):
                nc.scalar.dma_start(out=sphi_sb[:, pi * PIECE:(pi + 1) * PIECE],
                                    in_=d_sphi[:, pi * PIECE:(pi + 1) * PIECE])

            ansT = pp.tile([F, NSLOT], f32)
            aT = pp.tile([F, NSLOT], bf16)
            nc.sync.dma_start(out=aT[:], in_=d_x0T[:])

            for l in range(N_LAYERS):
                if l > 0:
                    nc.gpsimd.collective_compute(
                        "AllGather", mybir.AluOpType.bypass,
                        replica_groups=[list(range(NCORE))],
                        ins=[d_ash[:]], outs=[d_afull0[:]])
                    nc.sync.dma_start(out=d_apad[:NROWS // 2, 0:F],
                                      in_=d_afull0[:NROWS // 2, :])
                    nc.scalar.dma_start(out=d_apad[NROWS // 2:, 0:F],
                                        in_=d_afull0[NROWS // 2:, :])
                src = d_x0pad if l == 0 else d_apad

                NIC = GCHUNK * GE             # gather idxs per chunk (1024)
                for ch in range(NCHUNK):
                    if ch >= NCHUNK - SUP:
                        # last super: the PE idles here waiting for the final
                        # gather drains; junk matmuls keep HAM at K=8/8 so the
                        # serial end-of-layer tail (node GEMM + stores) runs
                        # at 2.4GHz instead of 1.2
                        pw = pout.tile([F, 512], f32, name="pw", tag="po0")
                        for _ in range(3):
                            nc.tensor.matmul(out=pw[:], lhsT=id_bf[:],
                                             rhs=aT[:, 0:512],
                                             start=True, stop=True)
                    xq = xp.tile([GE, GCHUNK * EP], bf16, tag="xj")
                    nc.gpsimd.dma_gather(
                        out_ap=xq[:].rearrange("p (g e) -> p g e", e=EP),
                        in_ap=src[:],
                        idxs_ap=gidx_sb[:, ch * NIC // 16:(ch + 1) * NIC // 16],
                        num_idxs=NIC, num_idxs_reg=NIC, elem_size=EP,
                        queue_num=ch % 4)
                    if ch % SUP == 0:
                        m_b = mp.tile([F, SUP * GCHUNK * GCOL], bf16, tag="mb")
                    for g4 in range(GCHUNK // 4):
                        ps = psc.tile([F, 4 * GCOL], f32, tag="psc")
                        for j in range(4):
                            gl = g4 * 4 + j
                            gg = ch * GCHUNK + gl
                            nc.tensor.matmul(
                                out=ps[:, j * GCOL:(j + 1) * GCOL],
                                lhsT=xq[:, gl * EP:gl * EP + F],
                                rhs=sphi_sb[:, gg * GCOL:(gg + 1) * GCOL],
                                start=True, stop=True)
                        qg = (ch % SUP) * GCHUNK + g4 * 4
                        mq = m_b[:].rearrange("c (b gg s) -> c gg b s",
                                              b=B, s=GS)
                        psv = ps[:].rearrange("c (g b s) -> c g b s",
                                              b=B, s=GS)
                        if g4 % 2 == 0:
                            nc.scalar.copy(
                                out=mq[:, qg:qg + 4], in_=psv[:])
                        else:
                            nc.vector.tensor_copy(
                                out=mq[:, qg:qg + 4], in_=psv[:])

                    if ch % SUP != SUP - 1:
                        continue
                    # ------- node-side GEMM for this super-batch -------
                    # 4 independent PSUM accumulation chains (b mod 4), so
                    # consecutive matmuls never hit the same PSUM region.
                    si = ch // SUP
                    sl = slice(si * NS, (si + 1) * NS)
                    # four full-bank chain tiles: PSUM accumulation groups
                    # cannot interleave within one bank zero-region
                    chains = [pout.tile([F, 512], f32, name=f"po{q}",
                                        tag=f"po{q}")
                              for q in range(4)]
                    for b in range(B):
                        q = b % 4
                        wofs = (l * B + b) * F
                        nc.tensor.matmul(
                            out=chains[q][:, 0:NS],
                            lhsT=convw_sb[:, wofs:wofs + F],
                            rhs=m_b[:, b * NS:(b + 1) * NS], start=(b < 4),
                            stop=(b >= B - 3))
                    nc.tensor.matmul(
                        out=chains[0][:, 0:NS],
                        lhsT=fcw_sb[:, l * F:(l + 1) * F],
                        rhs=aT[:, sl], start=False, stop=True)

                    # combine the 4 chains + bias (+ residual): one PSUM read
                    # per op (DVE cannot double-read one PSUM bank)
                    v0 = wp.tile([F, NS], f32, tag="v0")
                    nc.scalar.activation(
                        out=v0[:], in_=chains[0][:, 0:NS],
                        func=mybir.ActivationFunctionType.Identity,
                        bias=fcb_sb[:, l:l + 1])
                    v1 = wp.tile([F, NS], f32, tag="v1")
                    nc.vector.tensor_add(out=v1[:], in0=chains[1][:, 0:NS],
                                         in1=v0[:])
                    v2 = wp.tile([F, NS], f32, tag="v2")
                    nc.vector.tensor_add(out=v2[:], in0=chains[2][:, 0:NS],
                                         in1=v1[:])
                    if l == 0:
                        nc.vector.tensor_add(
                            out=ansT[:, sl], in0=chains[3][:, 0:NS],
                            in1=v2[:])
                    else:
                        v3 = wp.tile([F, NS], f32, tag="v3")
                        nc.vector.tensor_add(
                            out=v3[:], in0=chains[3][:, 0:NS], in1=v2[:])
                        nc.vector.tensor_add(
                            out=ansT[:, sl], in0=ansT[:, sl], in1=v3[:])

                    if l < N_LAYERS - 1:
                        nc.scalar.activation(
                            out=aT[:, sl], in_=ansT[:, sl],
                            func=mybir.ActivationFunctionType.Relu)
                        for k in range(NS // 128):
                            skl = slice(si * NS + k * 128,
                                        si * NS + (k + 1) * 128)
                            pt = ptr.tile([128, F], bf16, tag="ptrb")
                            nc.tensor.transpose(out=pt[:], in_=aT[:, skl],
                                                identity=id_bf[:])
                            aout = wp.tile([128, F], bf16, tag="aout")
                            nc.vector.tensor_copy(out=aout[:], in_=pt[:])
                            nc.sync.dma_start(out=d_ash[skl, :], in_=aout[:])
                    else:
                        oT = wp.tile([F, NS], f32, tag="oT")
                        nc.scalar.activation(
                            out=oT[:], in_=ansT[:, sl],
                            func=mybir.ActivationFunctionType.Copy,
                            scale=OUT_SCALE)
                        for k in range(NS // 128):
                            skl = slice(k * 128, (k + 1) * 128)
                            pt = ptr.tile([128, F], f32, tag="ptr")
                            nc.tensor.transpose(out=pt[:], in_=oT[:, skl],
                                                identity=id_f32[:])
                            oout = wp.tile([128, F], f32, tag="oout")
                            nc.vector.tensor_copy(out=oout[:], in_=pt[:])
                            nc.sync.dma_start(
                                out=d_out[si * NS + k * 128:
                                          si * NS + (k + 1) * 128, :],
                                in_=oout[:])

    nc.compile()

    # ------------------------------ run ------------------------------
    convw_c = np.ascontiguousarray(
        conv_w.transpose(2, 0, 1, 3).reshape(F, N_LAYERS * B * F)
    ).astype(ml_dtypes.bfloat16)
    fcw_c = np.ascontiguousarray(
        fc_w.transpose(1, 0, 2).reshape(F, N_LAYERS * F)).astype(ml_dtypes.bfloat16)
    fcb_T = np.ascontiguousarray(fc_b.T)
    x0_pad_bf = x0_pad.astype(ml_dtypes.bfloat16)
    sphi_bf = sphi.astype(ml_dtypes.bfloat16)

    in_maps = []
    for c in range(NCORE):
        x0T_c = np.ascontiguousarray(
            x0_slots[c * NSLOT:(c + 1) * NSLOT].T).astype(ml_dtypes.bfloat16)
        sphi_c = np.ascontiguousarray(
            sphi_bf[c].transpose(1, 0, 2).reshape(GE, NG * GCOL))
        in_maps.append({
            "x0pad": x0_pad_bf,
            "x0T": x0T_c,
            "sphi": sphi_c,
            "gidx": np.ascontiguousarray(idx_w[c]),
            "convw": convw_c,
            "fcw": fcw_c,
            "fcb": fcb_T,
        })

    import os
    global LAST_EXEC_NS, LAST_TRACE
    if os.environ.get("KERNEL_SIM"):
        from concourse.bass_interp import MultiCoreSim
        sim = MultiCoreSim(nc, num_cores=NCORE, require_finite=False,
                           require_nnan=False)
        for ci, core in sim.cores.items():
            for name, val in in_maps[ci].items():
                core.tensor(name)[:] = val
        sim.simulate(check_with_hw=False)
        out_slots = np.concatenate(
            [np.asarray(sim.cores[c].tensor("out")) for c in range(NCORE)], axis=0)
        LAST_EXEC_NS = None
        LAST_TRACE = None
        return out_slots[slot_of_node].astype(np.float32)

    res = run_bass_kernel_spmd(nc, in_maps, core_ids=list(range(NCORE)),
                               trace=PROFILE)
    LAST_EXEC_NS = res.exec_time_ns
    LAST_TRACE = res.instructions_and_trace[1] if res.instructions_and_trace else None
    out_slots = np.concatenate([res.results[c]["out"] for c in range(NCORE)], axis=0)
    return out_slots[slot_of_node].astype(np.float32)


def kernel(fluidFeatures, edge_i, edge_j, edge_attr, conv_ws, fc_ws, fc_bs):
    pre = _preprocess(edge_i, edge_j, edge_attr)
    return _build_and_run(pre, fluidFeatures, conv_ws, fc_ws, fc_bs)


# revision 25
# speedup vs baseline: 1.0250x; 1.0250x over previous
"""BasisNetwork (continuous-conv GNN) on 8 Trainium2 NeuronCores.

Per layer (nodes dest-sharded across cores, all compute in bf16/psum-f32):
  out[i] = sum_{e->i} (phi[e] (x) x[j_e]) @ Wflat  +  x[i] @ fc_w + fc_b
Edges are host-packed into groups of <=8 dest nodes / <=128 edge slots; a
static block-diagonal "Sphi" matrix (Sphi[k, b*8+slot] = phi[e_k, b] *
[dest(e_k)==slot], b-major columns) lives in SBUF for all 4 layers.  One
matmul per group with the gathered x_j tile as the stationary operand:
  M[c, b*8+slot] = sum_k xj[k, c] * Sphi[k, b*8+slot]
PSUM M-tiles are evicted into a b-major SBUF buffer (m_b[c, b*256 + g*8+s])
so the node-side GEMM per 32-group super-batch (256 slots) streams a fully
CONTIGUOUS moving operand (a strided rhs streams ~2x slower on the PE):
  out^T[f, slot] = sum_b conv_w[l][b].T @ m_b[:, b*256:(b+1)*256] (+fc+bias)
The 16 conv matmuls round-robin over 4 PSUM accumulation chains in four
SEPARATE PSUM banks: consecutive matmuls never target the same bank region
(same-region accumulation serializes at ~460ns/matmul, and interleaved
start/stop groups within one bank zero-region are illegal); chains are
summed on ACT/DVE with bias/residual fused.  Activations are all-gathered
(bf16) between layers and expanded into a 256B-strided table for GPSIMD
dma_gather.  Gathers use 512-index chunks round-robined over all 4 SWDGE
queues (each queue drains on ~one SDMA engine at ~27GiB/s — small chunks
keep 2-3 chunks resident per ring so drains run back-to-back) with a
24-deep xq tile pool so the gather stream never waits on compute.
"""

import numpy as np

# ---------------- problem constants (hardcoded per contract) ----------------
N_NODES = 20000
F = 32               # feature width, all layers
NB = 4
B = NB * NB          # 16 basis functions
N_LAYERS = 4
OUT_SCALE = 1.0 / 128.0
NCORE = 8
NSH = N_NODES // NCORE       # 2500 dest nodes per core
GS = 8                       # node slots per group
GE = 128                     # edge slots per group
GCOL = GS * B                # 128 columns per group (slot*16+b)
EP = 128                     # padded row length (256B in bf16) for dma_gather
GCHUNK = 4                   # groups per gather chunk (512 idxs)
SUP = 8                      # chunks per node-GEMM super-batch (256 slots)
PROFILE = False              # time warm executes
LAST_EXEC_NS = None
LAST_TRACE = None


def _hat(x, n):
    c = np.linspace(-1.0, 1.0, n, dtype=np.float32)
    r = np.abs(x[:, None] - c[None, :]) * ((n - 1) * 0.5)
    return np.maximum(1.0 - r, 0.0).astype(np.float32)


def _edge_basis(edge_attr, edge_i, edge_j):
    d = np.clip(edge_attr.astype(np.float32), -1.0, 1.0)
    phi = (_hat(d[:, 0], NB)[:, :, None] * _hat(d[:, 1], NB)[:, None, :]).reshape(-1, B)
    phi *= (edge_i != edge_j).astype(np.float32)[:, None]
    return phi  # [E, B]


def _pack_core(degrees, node_ids):
    """Best-fit-decreasing bin packing: bins of <=GS nodes / <=GE edges."""
    order = np.argsort(-degrees, kind="stable")
    bins = []            # list of [edges, [node,...]]
    buckets = [[] for _ in range(GE + 1)]
    for oi in order:
        d = int(degrees[oi])
        n = int(node_ids[oi])
        placed = False
        for r in range(d, GE + 1):
            while buckets[r]:
                bi = buckets[r][-1]
                e_cnt, nodes = bins[bi]
                if GE - e_cnt != r or len(nodes) >= GS:
                    buckets[r].pop()
                    continue
                nodes.append(n)
                bins[bi][0] = e_cnt + d
                buckets[r].pop()
                nr = GE - bins[bi][0]
                if len(nodes) < GS and nr > 0:
                    buckets[nr].append(bi)
                placed = True
                break
            if placed:
                break
        if not placed:
            bins.append([d, [n]])
            bi = len(bins) - 1
            nr = GE - d
            if nr > 0:
                buckets[nr].append(bi)
    return [nodes for _, nodes in bins]


def _preprocess(edge_i, edge_j, edge_attr):
    ei = np.asarray(edge_i).astype(np.int64)
    ej = np.asarray(edge_j).astype(np.int64)
    phi = _edge_basis(np.asarray(edge_attr), ei, ej)

    deg = np.bincount(ei, minlength=N_NODES)
    packs = []
    ng_max = 0
    for c in range(NCORE):
        nid = np.arange(c * NSH, (c + 1) * NSH)
        groups = _pack_core(deg[nid], nid)
        packs.append(groups)
        ng_max = max(ng_max, len(groups))

    NG = -(-ng_max // (GCHUNK * SUP)) * (GCHUNK * SUP)   # multiple of 64
    NSLOT = NG * GS
    assert NCORE * NSLOT < 32768    # int16 gather indices

    slot_of_node = np.zeros(N_NODES, dtype=np.int64)
    for c in range(NCORE):
        for gi, nodes in enumerate(packs[c]):
            for s, n in enumerate(nodes):
                slot_of_node[n] = c * NSLOT + gi * GS + s

    # per-core tables
    sphi = np.zeros((NCORE, NG, GE, GCOL), dtype=np.float32)
    gsrc = np.zeros((NCORE, GE, NG), dtype=np.int16)   # [edge slot p, group] -> src row
    order_all = np.argsort(ei, kind="stable")
    starts = np.zeros(N_NODES + 1, dtype=np.int64)
    np.cumsum(deg, out=starts[1:])
    for c in range(NCORE):
        for gi, nodes in enumerate(packs[c]):
            k = 0
            for s, n in enumerate(nodes):
                for e in order_all[starts[n]:starts[n + 1]]:
                    sphi[c, gi, k, s::GS] = phi[e]
                    gsrc[c, k, gi] = slot_of_node[ej[e]]
                    k += 1
            assert k <= GE
    # wrapped gather-index layout: stream k = gi*128 + p -> [k%16, k//16], x8 cores
    idx_w = np.zeros((NCORE, 16, NG * GE // 16), dtype=np.int16)
    for c in range(NCORE):
        stream = gsrc[c].T.reshape(-1)          # k = gi*128 + p
        idx_w[c, :, :] = stream.reshape(-1, 16).T
    idx_w = np.tile(idx_w, (1, 8, 1))           # [NCORE, 128, NG*8]

    return NG, NSLOT, slot_of_node, sphi, idx_w


def _build_and_run(pre, fluidFeatures, conv_ws, fc_ws, fc_bs):
    import ml_dtypes
    import concourse.bass as bass
    import concourse.bacc as bacc
    import concourse.mybir as mybir
    import concourse.tile as tile
    from concourse.bass_utils import run_bass_kernel_spmd
    from concourse.masks import make_identity
    from concourse import library_config

    NG, NSLOT, slot_of_node, sphi, idx_w = pre
    NCHUNK = NG // GCHUNK
    NSUPER = NCHUNK // SUP
    NROWS = NCORE * NSLOT
    NSPHI_PIECES = 8

    bf16 = mybir.dt.bfloat16
    f32 = mybir.dt.float32
    i16 = mybir.dt.int16

    x0 = np.asarray(fluidFeatures, dtype=np.float32)
    x0_slots = np.zeros((NROWS, F), dtype=np.float32)
    x0_slots[slot_of_node] = x0
    x0_pad = np.zeros((NROWS, EP), dtype=np.float32)
    x0_pad[:, :F] = x0_slots

    conv_w = np.stack([np.asarray(w, dtype=np.float32) for w in conv_ws])  # [L,B,F,F]
    fc_w = np.stack([np.asarray(w, dtype=np.float32) for w in fc_ws])      # [L,F,F]
    fc_b = np.stack([np.asarray(b, dtype=np.float32) for b in fc_bs])      # [L,F]

    NS = SUP * GCHUNK * GS          # slots per super (128)

    # ------------------------------ build graph ------------------------------
    nc = bacc.Bacc("TRN2", target_bir_lowering=False, debug=False, num_devices=NCORE,
                   num_swdge_queues=4)

    d_x0pad = nc.dram_tensor("x0pad", [NROWS, EP], bf16, kind="ExternalInput")
    d_x0T = nc.dram_tensor("x0T", [F, NSLOT], bf16, kind="ExternalInput")
    d_sphi = nc.dram_tensor("sphi", [GE, NG * GCOL], bf16, kind="ExternalInput")
    d_gidx = nc.dram_tensor("gidx", [128, NG * GE // 16], i16, kind="ExternalInput")
    d_convw = nc.dram_tensor("convw", [F, N_LAYERS * B * F], bf16, kind="ExternalInput")
    d_fcw = nc.dram_tensor("fcw", [F, N_LAYERS * F], bf16, kind="ExternalInput")
    d_fcb = nc.dram_tensor("fcb", [F, N_LAYERS], f32, kind="ExternalInput")
    d_out = nc.dram_tensor("out", [NSLOT, F], f32, kind="ExternalOutput")
    d_ash = nc.dram_tensor("a_shard", [NSLOT, F], bf16, kind="Internal")
    d_afull0 = nc.dram_tensor("a_full0", [NROWS, F], bf16,
                              kind="Internal", addr_space="Shared")
    d_apad = nc.dram_tensor("a_pad", [NROWS, EP], bf16, kind="Internal")

    with tile.TileContext(nc) as tc:
        with (
            tc.tile_pool(name="persist", bufs=1) as pp,
            tc.tile_pool(name="xj", bufs=24) as xp,
            tc.tile_pool(name="mb", bufs=2) as mp,
            tc.tile_pool(name="work", bufs=2) as wp,
            tc.tile_pool(name="psc", bufs=2, space="PSUM") as psc,
            tc.tile_pool(name="pout", bufs=1, space="PSUM") as pout,
            tc.tile_pool(name="ptr", bufs=1, space="PSUM") as ptr,
        ):
            nc.gpsimd.load_library(library_config.mlp)

            gidx_sb = pp.tile([128, NG * GE // 16], i16)
            nc.sync.dma_start(out=gidx_sb[:], in_=d_gidx[:])
            convw_sb = pp.tile([F, N_LAYERS * B * F], bf16)
            nc.sync.dma_start(out=convw_sb[:], in_=d_convw[:])
            fcw_sb = pp.tile([F, N_LAYERS * F], bf16)
            nc.sync.dma_start(out=fcw_sb[:], in_=d_fcw[:])
            fcb_sb = pp.tile([F, N_LAYERS], f32)
            nc.sync.dma_start(out=fcb_sb[:], in_=d_fcb[:])
            id_bf = pp.tile([F, F], bf16)
            make_identity(nc, id_bf[:])
            id_f32 = pp.tile([F, F], f32)
            make_identity(nc, id_f32[:])

            # resident Sphi: loaded once in pieces, reused by all layers
            sphi_sb = pp.tile([GE, NG * GCOL], bf16)
            PIECE = NG * GCOL // NSPHI_PIECES
            for pi in range(NSPHI_PIECES):
                nc.scalar.dma_start(out=sphi_sb[:, pi * PIECE:(pi + 1) * PIECE],
                                    in_=d_sphi[:, pi * PIECE:(pi + 1) * PIECE])

            ansT = pp.tile([F, NSLOT], f32)
            aT = pp.tile([F, NSLOT], bf16)
            nc.sync.dma_start(out=aT[:], in_=d_x0T[:])

            for l in range(N_LAYERS):
                if l > 0:
                    nc.gpsimd.collective_compute(
                        "AllGather", mybir.AluOpType.bypass,
                        replica_groups=[list(range(NCORE))],
                        ins=[d_ash[:]], outs=[d_afull0[:]])
                    nc.sync.dma_start(out=d_apad[:NROWS // 2, 0:F],
                                      in_=d_afull0[:NROWS // 2, :])
                    nc.scalar.dma_start(out=d_apad[NROWS // 2:, 0:F],
                                        in_=d_afull0[NROWS // 2:, :])
                src = d_x0pad if l == 0 else d_apad

                NIC = GCHUNK * GE             # gather idxs per chunk (1024)
                for ch in range(NCHUNK):
                    xq = xp.tile([GE, GCHUNK * EP], bf16, tag="xj")
                    nc.gpsimd.dma_gather(
                        out_ap=xq[:].rearrange("p (g e) -> p g e", e=EP),
                        in_ap=src[:],
                        idxs_ap=gidx_sb[:, ch * NIC // 16:(ch + 1) * NIC // 16],
                        num_idxs=NIC, num_idxs_reg=NIC, elem_size=EP,
                        queue_num=ch % 4)
                    if ch % SUP == 0:
                        m_b = mp.tile([F, SUP * GCHUNK * GCOL], bf16, tag="mb")
                    for g4 in range(GCHUNK // 4):
                        ps = psc.tile([F, 4 * GCOL], f32, tag="psc")
                        for j in range(4):
                            gl = g4 * 4 + j
                            gg = ch * GCHUNK + gl
                            nc.tensor.matmul(
                                out=ps[:, j * GCOL:(j + 1) * GCOL],
                                lhsT=xq[:, gl * EP:gl * EP + F],
                                rhs=sphi_sb[:, gg * GCOL:(gg + 1) * GCOL],
                                start=True, stop=True)
                        qg = (ch % SUP) * GCHUNK + g4 * 4
                        mq = m_b[:].rearrange("c (b gg s) -> c gg b s",
                                              b=B, s=GS)
                        psv = ps[:].rearrange("c (g b s) -> c g b s",
                                              b=B, s=GS)
                        if g4 % 2 == 0:
                            nc.scalar.copy(
                                out=mq[:, qg:qg + 4], in_=psv[:])
                        else:
                            nc.vector.tensor_copy(
                                out=mq[:, qg:qg + 4], in_=psv[:])

                    if ch % SUP != SUP - 1:
                        continue
                    # ------- node-side GEMM for this super-batch -------
                    # 4 independent PSUM accumulation chains (b mod 4), so
                    # consecutive matmuls never hit the same PSUM region.
                    si = ch // SUP
                    sl = slice(si * NS, (si + 1) * NS)
                    # four full-bank chain tiles: PSUM accumulation groups
                    # cannot interleave within one bank zero-region
                    chains = [pout.tile([F, 512], f32, name=f"po{q}",
                                        tag=f"po{q}")
                              for q in range(4)]
                    for b in range(B):
                        q = b % 4
                        wofs = (l * B + b) * F
                        nc.tensor.matmul(
                            out=chains[q][:, 0:NS],
                            lhsT=convw_sb[:, wofs:wofs + F],
                            rhs=m_b[:, b * NS:(b + 1) * NS], start=(b < 4),
                            stop=(b >= B - 3))
                    nc.tensor.matmul(
                        out=chains[0][:, 0:NS],
                        lhsT=fcw_sb[:, l * F:(l + 1) * F],
                        rhs=aT[:, sl], start=False, stop=True)

                    # combine the 4 chains + bias (+ residual): one PSUM read
                    # per op (DVE cannot double-read one PSUM bank)
                    v0 = wp.tile([F, NS], f32, tag="v0")
                    nc.scalar.activation(
                        out=v0[:], in_=chains[0][:, 0:NS],
                        func=mybir.ActivationFunctionType.Identity,
                        bias=fcb_sb[:, l:l + 1])
                    v1 = wp.tile([F, NS], f32, tag="v1")
                    nc.vector.tensor_add(out=v1[:], in0=chains[1][:, 0:NS],
                                         in1=v0[:])
                    v2 = wp.tile([F, NS], f32, tag="v2")
                    nc.vector.tensor_add(out=v2[:], in0=chains[2][:, 0:NS],
                                         in1=v1[:])
                    if l == 0:
                        nc.vector.tensor_add(
                            out=ansT[:, sl], in0=chains[3][:, 0:NS],
                            in1=v2[:])
                    else:
                        v3 = wp.tile([F, NS], f32, tag="v3")
                        nc.vector.tensor_add(
                            out=v3[:], in0=chains[3][:, 0:NS], in1=v2[:])
                        nc.vector.tensor_add(
                            out=ansT[:, sl], in0=ansT[:, sl], in1=v3[:])

                    if l < N_LAYERS - 1:
                        nc.scalar.activation(
                            out=aT[:, sl], in_=ansT[:, sl],
                            func=mybir.ActivationFunctionType.Relu)
                        for k in range(NS // 128):
                            skl = slice(si * NS + k * 128,
                                        si * NS + (k + 1) * 128)
                            pt = ptr.tile([128, F], bf16, tag="ptrb")
                            nc.tensor.transpose(out=pt[:], in_=aT[:, skl],
                                                identity=id_bf[:])
                            aout = wp.tile([128, F], bf16, tag="aout")
                            nc.vector.tensor_copy(out=aout[:], in_=pt[:])
                            nc.sync.dma_start(out=d_ash[skl, :], in_=aout[:])
                    else:
                        oT = wp.tile([F, NS], f32, tag="oT")
                        nc.scalar.activation(
                            out=oT[:], in_=ansT[:, sl],
                            func=mybir.ActivationFunctionType.Copy,
                            scale=OUT_SCALE)
                        for k in range(NS // 128):
                            skl = slice(k * 128, (k + 1) * 128)
                            pt = ptr.tile([128, F], f32, tag="ptr")
                            nc.tensor.transpose(out=pt[:], in_=oT[:, skl],
                                                identity=id_f32[:])
                            oout = wp.tile([128, F], f32, tag="oout")
                            nc.vector.tensor_copy(out=oout[:], in_=pt[:])
                            nc.sync.dma_start(
                                out=d_out[si * NS + k * 128:
                                          si * NS + (k + 1) * 128, :],
                                in_=oout[:])

    nc.compile()

    # ------------------------------ run ------------------------------
    convw_c = np.ascontiguousarray(
        conv_w.transpose(2, 0, 1, 3).reshape(F, N_LAYERS * B * F)
    ).astype(ml_dtypes.bfloat16)
    fcw_c = np.ascontiguousarray(
        fc_w.transpose(1, 0, 2).reshape(F, N_LAYERS * F)).astype(ml_dtypes.bfloat16)
    fcb_T = np.ascontiguousarray(fc_b.T)
    x0_pad_bf = x0_pad.astype(ml_dtypes.bfloat16)
    sphi_bf = sphi.astype(ml_dtypes.bfloat16)

    in_maps = []
    for c in range(NCORE):
        x0T_c = np.ascontiguousarray(
            x0_slots[c * NSLOT:(c + 1) * NSLOT].T).astype(ml_dtypes.bfloat16)
        sphi_c = np.ascontiguousarray(
            sphi_bf[c].transpose(1, 0, 2).reshape(GE, NG * GCOL))
        in_maps.append({
            "x0pad": x0_pad_bf,
            "x0T": x0T_c,
            "sphi": sphi_c,
            "gidx": np.ascontiguousarray(idx_w[c]),
            "convw": convw_c,
            "fcw": fcw_c,
            "fcb": fcb_T,
        })

    import os
    global LAST_EXEC_NS, LAST_TRACE
    if os.environ.get("KERNEL_SIM"):
        from concourse.bass_interp import MultiCoreSim
        sim = MultiCoreSim(nc, num_cores=NCORE, require_finite=False,
                           require_nnan=False)
        for ci, core in sim.cores.items():
            for name, val in in_maps[ci].items():
                core.tensor(name)[:] = val
        sim.simulate(check_with_hw=False)
        out_slots = np.concatenate(
            [np.asarray(sim.cores[c].tensor("out")) for c in range(NCORE)], axis=0)
        LAST_EXEC_NS = None
        LAST_TRACE = None
        return out_slots[slot_of_node].astype(np.float32)

    res = run_bass_kernel_spmd(nc, in_maps, core_ids=list(range(NCORE)),
                               trace=PROFILE)
    LAST_EXEC_NS = res.exec_time_ns
    LAST_TRACE = res.instructions_and_trace[1] if res.instructions_and_trace else None
    out_slots = np.concatenate([res.results[c]["out"] for c in range(NCORE)], axis=0)
    return out_slots[slot_of_node].astype(np.float32)


def kernel(fluidFeatures, edge_i, edge_j, edge_attr, conv_ws, fc_ws, fc_bs):
    pre = _preprocess(edge_i, edge_j, edge_attr)
    return _build_and_run(pre, fluidFeatures, conv_ws, fc_ws, fc_bs)
